# revision 3
# baseline (speedup 1.0000x reference)
"""Trainium2 Bass kernel for nn_Base_Filter — banded-Toeplitz matmul scheme.

Math (folded on host):
  w2  = 0.9*(1+w_p) * g*v/||v||_F      per-channel effective 7x7 kernel
  c2  = 0.9*b_p*sum(g*v/||v||_F)       per-channel bias
  out = lrelu(depthwise_conv7x7_valid(x, w2) + c2, alpha=1/90)

Device scheme (per core, 128 channels, one channel at a time on TensorE):
  - Layout: partitions = image ROWS of one channel (not channels).
  - Per channel: two 128-row strips (rows 0..127 -> out rows 0..121, rows
    122..249 -> out rows 122..243) side by side in one SBUF tile [128,2,256].
  - 7 matmuls (one per kernel column dj) accumulate into PSUM [122, 2, 250]:
      lhsT[pi, po] = w2[ch, pi-po, dj]   (banded Toeplitz, host-built)
      rhs          = xs[:, :, dj:dj+250]
    Each matmul contracts all 7 row-taps at once, so a channel costs
    7 x 500 PE-rows instead of 49 x 500 on the per-tap diagonal scheme.
    This is the PE lower bound for this conv: out rows per strip are capped
    at 122 (128 partitions - 6 band overlap) and the 7 column shifts cannot
    be merged into one matmul (free-dim shifts are not expressible).
  - ScalarE applies Lrelu(+c2) while evacuating PSUM -> SBUF bf16; stores
    ride the otherwise-idle Pool engine's SWDGE queue (1000B chunks) so the
    SP queue streams loads without ever blocking on a store's semaphore.
  - Out rows 244..249 (the strip remainder) are computed by VectorE in the
    channels-on-partitions layout (49 scalar_tensor_tensor taps) entirely
    hidden under TensorE; its epilogue is emitted mid-loop, off both ends
    of the critical path.
  - All wire traffic is bf16 (x strips, lhsT bands, outputs); fine-grained
    2-channel DMA quanta + deep pools (bufs 6-10) keep the exclusive DMA
    device smooth at ~88% busy under TensorE's 95%.
TimelineSim: 196.0us/core (PE busy 186.8us = floor; baseline was 897.4us).
HW-verified rel err 3.1e-3 (scale-rel absmax 3.9e-3).
"""

import os
import numpy as np
import ml_dtypes

BF16 = np.dtype(ml_dtypes.bfloat16)

A = 256
B = 256
R = 32
C = 32
K = 1024
KS = 7
NCORES = 8
P = 128          # channels per core
AO = A - KS + 1  # 250
BO = B - KS + 1  # 250
SO = 122         # out rows per strip (128 - 6)
NS = 2           # strips per channel
TAIL = AO - NS * SO  # 6 tail rows

XG = 2           # channels per x DMA
WG = 2           # channels per lhsT DMA
OG = 2           # channels per out DMA

_COMPILED = {}
LAST_RESULTS = None  # BassKernelResults of the most recent run (for test.py)


def _build_nc():
    import concourse.bacc as bacc
    import concourse.mybir as mybir
    import concourse.tile as tile

    f32 = mybir.dt.float32
    bf16 = mybir.dt.bfloat16
    nc = bacc.Bacc("TRN2", target_bir_lowering=False, debug=False, num_devices=NCORES)

    x_d = nc.declare_dram_parameter("x", [P, P, NS, B], bf16, isOutput=False)
    w_d = nc.declare_dram_parameter("w", [P, P, KS, SO], bf16, isOutput=False)
    c2b_d = nc.declare_dram_parameter("c2b", [P, P], f32, isOutput=False)
    c2t_d = nc.declare_dram_parameter("c2t", [P, 1], f32, isOutput=False)
    wv_d = nc.declare_dram_parameter("wv", [P, KS * KS], f32, isOutput=False)
    xt_d = nc.declare_dram_parameter("xt", [P, TAIL + KS - 1, B], bf16, isOutput=False)
    out_d = nc.declare_dram_parameter("out", [SO, P, NS, BO], bf16, isOutput=True)
    outt_d = nc.declare_dram_parameter("outt", [P, TAIL, BO], bf16, isOutput=True)

    taps = [(di, dj) for di in range(KS) for dj in range(KS)]

    with tile.TileContext(nc) as tc:
        from contextlib import ExitStack

        with ExitStack() as ctx:
            const = ctx.enter_context(tc.tile_pool(name="const", bufs=1))
            xpool = ctx.enter_context(tc.tile_pool(name="x", bufs=10))
            wpool = ctx.enter_context(tc.tile_pool(name="w", bufs=10))
            opool = ctx.enter_context(tc.tile_pool(name="o", bufs=6))
            tpool = ctx.enter_context(tc.tile_pool(name="tl", bufs=1))
            ppool = ctx.enter_context(tc.tile_pool(name="ps", bufs=6, space="PSUM"))

            # prologue: strictly critical-first per-channel loads (chs 0-5)
            # so the PE pipeline fills at the SP issue rate with no waste
            xs0 = xpool.tile([P, XG, NS, B], bf16, tag="xs")
            wt0 = wpool.tile([P, WG, KS, SO], bf16, tag="wt")
            nc.sync.dma_start(wt0[:, 0:1, 0:2], w_d[:, 0:1, 0:2, :])
            nc.sync.dma_start(xs0[:, 0:1], x_d[:, 0:1, :, :])
            nc.sync.dma_start(wt0[:, 0:1, 2:KS], w_d[:, 0:1, 2:KS, :])
            nc.sync.dma_start(wt0[:, 1:WG], w_d[:, 1:WG, :, :])
            nc.sync.dma_start(xs0[:, 1:XG], x_d[:, 1:XG, :, :])
            c2b_sb = const.tile([P, P], f32)
            nc.sync.dma_start(c2b_sb[:], c2b_d[:])
            xs1 = xpool.tile([P, XG, NS, B], bf16, tag="xs")
            wt1 = wpool.tile([P, WG, KS, SO], bf16, tag="wt")
            nc.sync.dma_start(wt1[:, 0:1], w_d[:, 2:3, :, :])
            nc.sync.dma_start(xs1[:, 0:1], x_d[:, 2:3, :, :])
            nc.sync.dma_start(wt1[:, 1:WG], w_d[:, 3:4, :, :])
            nc.sync.dma_start(xs1[:, 1:XG], x_d[:, 3:4, :, :])
            xs2 = xpool.tile([P, XG, NS, B], bf16, tag="xs")
            wt2 = wpool.tile([P, WG, KS, SO], bf16, tag="wt")
            nc.sync.dma_start(wt2[:, 0:1], w_d[:, 4:5, :, :])
            nc.sync.dma_start(xs2[:, 0:1], x_d[:, 4:5, :, :])
            nc.sync.dma_start(wt2[:, 1:WG], w_d[:, 5:6, :, :])
            nc.sync.dma_start(xs2[:, 1:XG], x_d[:, 5:6, :, :])
            xs3 = xpool.tile([P, XG, NS, B], bf16, tag="xs")
            wt3 = wpool.tile([P, WG, KS, SO], bf16, tag="wt")
            nc.sync.dma_start(wt3[:], w_d[:, 6:8, :, :])
            nc.sync.dma_start(xs3[:], x_d[:, 6:8, :, :])
            c2t_sb = const.tile([P, 1], f32)
            wv_sb = const.tile([P, KS * KS], f32)
            xt_sb = tpool.tile([P, TAIL + KS - 1, B], bf16)
            tacc = tpool.tile([P, TAIL, BO], f32)

            # ---- main: per-channel banded-Toeplitz matmuls ----
            outs = None
            xs, wt = xs0, wt0
            for ch in range(P):
                if ch == 2:
                    xs, wt = xs1, wt1
                elif ch == 4:
                    xs, wt = xs2, wt2
                elif ch == 6:
                    xs, wt = xs3, wt3
                elif ch % XG == 0 and ch >= 8:
                    xs = xpool.tile([P, XG, NS, B], bf16, tag="xs")
                    nc.sync.dma_start(xs[:], x_d[:, ch : ch + XG, :, :])
                    wt = wpool.tile([P, WG, KS, SO], bf16, tag="wt")
                    nc.sync.dma_start(wt[:], w_d[:, ch : ch + WG, :, :])
                if ch == 16:
                    nc.sync.dma_start(c2t_sb[:], c2t_d[:])
                    nc.sync.dma_start(wv_sb[:], wv_d[:])
                if ch % OG == 0:
                    outs = opool.tile([SO, OG, NS, BO], bf16, tag="outs")
                if ch == 8:
                    nc.sync.dma_start(xt_sb[:, 0:6, :], xt_d[:, 0:6, :])
                if ch == 20:
                    # tail input + DVE tap chain, deferred + split so the
                    # 12KB/partition load never backlogs the DMA device;
                    # DVE has ~100us of slack so the late start is free
                    nc.sync.dma_start(xt_sb[:, 6:, :], xt_d[:, 6:, :])
                    for j, (di, dj) in enumerate(taps):
                        rhs = xt_sb[:, di : di + TAIL, dj : dj + BO]
                        if j == 0:
                            nc.vector.tensor_scalar(
                                tacc[:], rhs, wv_sb[:, 0:1], None,
                                mybir.AluOpType.mult,
                            )
                        else:
                            nc.vector.scalar_tensor_tensor(
                                tacc[:],
                                rhs,
                                wv_sb[:, j : j + 1],
                                tacc[:],
                                mybir.AluOpType.mult,
                                mybir.AluOpType.add,
                            )

                ps = ppool.tile([SO, NS, BO], f32, tag="ps")
                for dj in range(KS):
                    nc.tensor.matmul(
                        ps[:],
                        wt[:, ch % WG, dj, :],
                        xs[:, ch % XG, :, dj : dj + BO],
                        start=(dj == 0),
                        stop=(dj == KS - 1),
                    )
                nc.scalar.activation(
                    outs[:, ch % OG, :, :],
                    ps[:],
                    mybir.ActivationFunctionType.Lrelu,
                    bias=c2b_sb[0:SO, ch : ch + 1],
                    scale=1.0,
                    alpha=0.01 / 0.9,
                )
                if ch == P - 2:
                    # taper: last two stores ride SP, which has drained its
                    # loads by now, avoiding the Pool store-backlog latency
                    nc.sync.dma_start(
                        out_d[:, P - 2 : P - 1, :, :], outs[:, 0:1]
                    )
                elif ch == P - 1:
                    nc.sync.dma_start(
                        out_d[:, P - 1 : P, :, :], outs[:, 1:2]
                    )
                elif ch % OG == OG - 1:
                    # stores ride the idle Pool engine's SWDGE queue so they
                    # never block SP's in-order load stream, and their sems
                    # are satisfied when Pool reaches them
                    nc.gpsimd.dma_start(
                        out_d[:, ch - OG + 1 : ch + 1, :, :], outs[:]
                    )
                if ch == 88:
                    # tail epilogue act: tacc is long done; ACT slack is wide
                    tout = tpool.tile([P, TAIL, BO], bf16)
                    nc.scalar.activation(
                        tout[:],
                        tacc[:],
                        mybir.ActivationFunctionType.Lrelu,
                        bias=c2t_sb[:, 0:1],
                        scale=1.0,
                        alpha=0.01 / 0.9,
                    )
                if ch == 92:
                    # by now tout is ready, so this never blocks the store queue
                    nc.gpsimd.dma_start(outt_d[:], tout[:])

    nc.compile()
    return nc


def _prep_weights(w_p, b_p, v, g):
    v = v.astype(np.float32)
    v_norm = np.sqrt((v * v).sum(axis=(1, 2), keepdims=True))
    w_eff = g[:, None, None].astype(np.float32) * v / v_norm          # [K,7,7]
    w2 = 0.9 * (1.0 + w_p)[:, None, None].astype(np.float32) * w_eff  # [K,7,7]
    c2 = (0.9 * b_p.astype(np.float32) * w_eff.sum(axis=(1, 2)))      # [K]
    return w2.astype(np.float32), c2.astype(np.float32)


def kernel(x, w_p, b_p, v, g):
    global LAST_RESULTS
    from concourse.bass_utils import run_bass_kernel_spmd

    x = np.asarray(x, dtype=np.float32)
    w2, c2 = _prep_weights(
        np.asarray(w_p, np.float32),
        np.asarray(b_p, np.float32),
        np.asarray(v, np.float32),
        np.asarray(g, np.float32),
    )

    # channel-major x: [K, A, B], k = r*C + c (matches reference kernel_index)
    x_t = np.ascontiguousarray(x.transpose(2, 3, 0, 1).reshape(K, A, B))
    x_bf = x_t.astype(BF16)

    apo = np.arange(SO)
    in_maps = []
    for core in range(NCORES):
        sl = slice(core * P, (core + 1) * P)
        xc = x_bf[sl]                       # [P, A, B] bf16
        w2c = w2[sl]                        # [P, 7, 7]

        # x strips, partition(row)-major: [p, ch, strip, col]
        xs_host = np.empty((P, P, NS, B), dtype=BF16)
        xs_host[:, :, 0, :] = xc[:, 0:P, :].transpose(1, 0, 2)
        xs_host[:, :, 1, :] = xc[:, AO - P : AO, :].transpose(1, 0, 2)

        # banded lhsT: lhsT[pi, ch, dj, po] = w2c[ch, pi-po, dj]
        wt_f32 = np.zeros((P, P, KS, SO), dtype=np.float32)  # [ch, pi, dj, po]
        for di in range(KS):
            wt_f32[:, apo + di, :, apo] = w2c[:, di, :][None]
        wt_host = np.ascontiguousarray(wt_f32.transpose(1, 0, 2, 3)).astype(BF16)

        c2c = c2[sl]
        in_maps.append(
            {
                "x": xs_host,
                "w": wt_host,
                "c2b": np.ascontiguousarray(
                    np.broadcast_to(c2c[None, :], (P, P)).astype(np.float32)
                ),
                "c2t": np.ascontiguousarray(c2c[:, None]),
                "wv": np.ascontiguousarray(w2c.reshape(P, KS * KS)),
                "xt": np.ascontiguousarray(
                    xc[:, NS * SO : NS * SO + TAIL + KS - 1, :]
                ),
            }
        )

    key = "toeplitz_v2"
    if key not in _COMPILED:
        _COMPILED[key] = _build_nc()
    nc = _COMPILED[key]

    trace = os.environ.get("KRN_TRACE", "0") == "1"
    res = run_bass_kernel_spmd(nc, in_maps, list(range(NCORES)), trace=trace)
    LAST_RESULTS = res

    out_full = np.empty((K, AO, BO), dtype=np.float32)
    for core in range(NCORES):
        sl = slice(core * P, (core + 1) * P)
        main = res.results[core]["out"]     # [SO, P, NS, BO] bf16
        tail = res.results[core]["outt"]    # [P, TAIL, BO] bf16
        out_full[sl, 0 : NS * SO] = (
            main.transpose(1, 2, 0, 3).reshape(P, NS * SO, BO).astype(np.float32)
        )
        out_full[sl, NS * SO :] = tail.astype(np.float32)

    # [K, AO, BO] -> [AO, BO, R, C]
    return np.ascontiguousarray(
        out_full.reshape(R, C, AO, BO).transpose(2, 3, 0, 1)
    )


def get_nc():
    return _COMPILED.get("toeplitz_v2")


if __name__ == "__main__":
    rng = np.random.default_rng(0)
    xs = rng.standard_normal((A, B, R, C), dtype=np.float32)
    out = kernel(
        xs,
        rng.standard_normal(K).astype(np.float32) * 0.1,
        rng.standard_normal(K).astype(np.float32) * 0.1,
        rng.standard_normal((K, KS, KS)).astype(np.float32),
        rng.standard_normal(K).astype(np.float32),
    )
    print(out.shape, out.dtype)
